# revision 5
# baseline (speedup 1.0000x reference)
"""DemopackDecoder Trainium2 kernel (8 NeuronCores, tensor-parallel).

Problem:
    weight = concat_t[ (codewords[indices[t]] @ rotations[t]) * scales[t] ]   # [4096, 4096]
    out    = x @ weight.T + bias                                              # [4, 2048, 4096]

Sharding: out_features (4096 = 4 tiles x 1024 rows) split across 8 cores,
512 rows each (core d -> tile t=d//2, half h=d%2). x is replicated; each core
computes its 512 output columns; host concatenates.

Per-core device program (all matmuls in float32r = full-speed fp32 PE mode):
  phase 1:  WT[e, r] = sum_d R[d, e] * CT[d, r]      (CT = scaled gathered
            codewords, transposed on host; R = rotation tile) -> WT resident
            in SBUF as [128, 32, 512]
  phase 2:  O[s, o] = sum_e XT[e, s] * WT[e, o]      (XT = x^T, streamed from
            HBM as stationary blocks; WT is the moving operand from SBUF)

Host does: transpose of x (layout prep), codeword gather + transpose + scale
fold (8.4 MB per core), bias add (bias is zeros in this problem, kept for
generality).
"""

import time

import numpy as np

import concourse.mybir as mybir
from concourse import bacc, tile

F32 = mybir.dt.float32
F32R = mybir.dt.float32r

D = 4096          # embed dim == in_features (contraction for both phases)
S = 8192          # B * S tokens
O_PER = 512       # out_features per core
N_CORES = 8

DO = D // 128     # 32 contraction chunks
P = 128

_CACHE = {}


def _build():
    nc = bacc.Bacc("TRN2", target_bir_lowering=False, debug=False,
                   num_devices=N_CORES)
    xt = nc.dram_tensor("xt", [D, S], F32R, kind="ExternalInput").ap()
    rot = nc.dram_tensor("rot", [D, D], F32R, kind="ExternalInput").ap()
    ct = nc.dram_tensor("ct", [D, O_PER], F32R, kind="ExternalInput").ap()
    out = nc.dram_tensor("out", [S, O_PER], F32, kind="ExternalOutput").ap()

    ct_r = ct.rearrange("(do p) r -> p do r", p=P)

    with tile.TileContext(nc) as tc:
        with (
            tc.tile_pool(name="resident", bufs=1) as resident,
            tc.tile_pool(name="rx", bufs=4) as rx,
            tc.tile_pool(name="outp", bufs=6) as outp,
            tc.tile_pool(name="ps", bufs=8, space="PSUM") as ps,
        ):
            ct_sb = resident.tile([P, DO, O_PER], F32R)
            wt_sb = resident.tile([P, DO, O_PER], F32R)

            for do in range(DO):
                nc.sync.dma_start(out=ct_sb[:, do, :], in_=ct_r[:, do, :])

            # ---- phase 1: WT = R^T-blocks x CT  (out e-partitions) ----
            for eg in range(8):          # groups of 4 e-tiles of 128
                psums = [
                    ps.tile([P, O_PER], F32, name=f"ps1_{eg}_{j}", tag="ps")
                    for j in range(4)
                ]
                for do in range(DO):
                    rt = rx.tile([P, 512], F32R, name="rt", tag="rt", bufs=4)
                    nc.sync.dma_start(
                        out=rt[:],
                        in_=rot[do * P:(do + 1) * P, eg * 512:(eg + 1) * 512],
                    )
                    for j in range(4):
                        nc.tensor.matmul(
                            psums[j][:],
                            lhsT=rt[:, j * P:(j + 1) * P],
                            rhs=ct_sb[:, do, :],
                            start=(do == 0),
                            stop=(do == DO - 1),
                        )
                for j in range(4):
                    nc.scalar.copy(wt_sb[:, eg * 4 + j, :], psums[j][:])

            # ---- phase 2: O = XT-blocks x WT  (out s-partitions) ----
            for sg in range(16):         # groups of 4 s-tiles of 128
                psums = [
                    ps.tile([P, O_PER], F32, name=f"ps2_{sg}_{j}", tag="ps")
                    for j in range(4)
                ]
                for eo in range(DO):
                    xtl = rx.tile([P, 512], F32R, name="xtl", tag="xtl", bufs=6)
                    nc.sync.dma_start(
                        out=xtl[:],
                        in_=xt[eo * P:(eo + 1) * P, sg * 512:(sg + 1) * 512],
                    )
                    for j in range(4):
                        nc.tensor.matmul(
                            psums[j][:],
                            lhsT=xtl[:, j * P:(j + 1) * P],
                            rhs=wt_sb[:, eo, :],
                            start=(eo == 0),
                            stop=(eo == DO - 1),
                        )
                for j in range(4):
                    st = sg * 4 + j
                    ot = outp.tile([P, O_PER], F32, name="ot", tag="ot")
                    nc.scalar.copy(ot[:], psums[j][:])
                    nc.sync.dma_start(
                        out=out[st * P:(st + 1) * P, :], in_=ot[:]
                    )

    nc.compile()
    return nc


class _Runner:
    """Compile once; execute the SPMD NEFF via PJRT shard_map repeatedly."""

    def __init__(self):
        import jax
        from jax.experimental.shard_map import shard_map
        from jax.sharding import Mesh, NamedSharding, PartitionSpec

        from concourse.bass2jax import (
            _bass_exec_p,
            install_neuronx_cc_hook,
            partition_id_tensor,
        )

        self.jax = jax
        install_neuronx_cc_hook()
        nc = _build()
        self.nc = nc

        in_names: list[str] = []
        out_names: list[str] = []
        out_avals: list = []
        zero_shapes: list = []
        partition_name = (
            nc.partition_id_tensor.name if nc.partition_id_tensor else None
        )
        for alloc in nc.m.functions[0].allocations:
            if not isinstance(alloc, mybir.MemoryLocationSet):
                continue
            name = alloc.memorylocations[0].name
            if alloc.kind == "ExternalInput":
                if name != partition_name:
                    in_names.append(name)
            elif alloc.kind == "ExternalOutput":
                np_dt = mybir.dt.np(alloc.dtype)
                out_names.append(name)
                out_avals.append(
                    jax.core.ShapedArray(tuple(alloc.tensor_shape), np_dt)
                )
                zero_shapes.append((tuple(alloc.tensor_shape), np_dt))

        self.n_params = len(in_names)
        self.in_names = list(in_names)
        self.out_names = list(out_names)
        self.out_avals = out_avals
        self.zero_shapes = zero_shapes

        all_in_names = in_names + out_names
        if partition_name is not None:
            all_in_names = all_in_names + [partition_name]

        def _body(*args):
            operands = list(args)
            if partition_name is not None:
                operands.append(partition_id_tensor())
            outs = _bass_exec_p.bind(
                *operands,
                out_avals=tuple(out_avals),
                in_names=tuple(all_in_names),
                out_names=tuple(out_names),
                lowering_input_output_aliases=(),
                sim_require_finite=True,
                sim_require_nnan=True,
                nc=nc,
            )
            return tuple(outs)

        devices = jax.devices()[:N_CORES]
        assert len(devices) == N_CORES
        self.mesh = Mesh(np.asarray(devices), ("core",))
        n_args = self.n_params + len(out_names)
        self.fn = jax.jit(
            shard_map(
                _body,
                mesh=self.mesh,
                in_specs=(PartitionSpec("core"),) * n_args,
                out_specs=(PartitionSpec("core"),) * len(out_names),
                check_rep=False,
            ),
            keep_unused=True,
        )
        self.sharding = NamedSharding(self.mesh, PartitionSpec("core"))
        self.dev_args = None

    def put_inputs(self, in_maps):
        jax = self.jax
        args = []
        for i, name in enumerate(self.in_names):
            cat = np.concatenate([np.asarray(m[name]) for m in in_maps], axis=0)
            args.append(jax.device_put(cat, self.sharding))
        for shape, np_dt in self.zero_shapes:
            z = np.zeros((N_CORES * shape[0], *shape[1:]), np_dt)
            args.append(jax.device_put(z, self.sharding))
        self.dev_args = args

    def run(self):
        jax = self.jax
        outs = self.fn(*self.dev_args)
        jax.block_until_ready(outs)
        res = []
        for c in range(N_CORES):
            res.append({
                name: np.asarray(outs[i]).reshape(
                    N_CORES, *self.out_avals[i].shape
                )[c]
                for i, name in enumerate(self.out_names)
            })
        return res

    def bench(self, iters=10):
        jax = self.jax
        outs = self.fn(*self.dev_args)
        jax.block_until_ready(outs)
        t0 = time.perf_counter()
        for _ in range(iters):
            outs = self.fn(*self.dev_args)
        jax.block_until_ready(outs)
        dt = (time.perf_counter() - t0) / iters
        return dt


def _get_runner():
    if "runner" not in _CACHE:
        _CACHE["runner"] = _Runner()
    return _CACHE["runner"]


def kernel(x, codewords, indices, rotations, scales, bias):
    x = np.asarray(x, dtype=np.float32)
    codewords = np.asarray(codewords, dtype=np.float32)
    indices = np.asarray(indices)
    rotations = np.asarray(rotations, dtype=np.float32)
    scales = np.asarray(scales, dtype=np.float32)
    bias = np.asarray(bias, dtype=np.float32)

    runner = _get_runner()

    xt = np.ascontiguousarray(x.reshape(S, D).T)  # [D, S]

    in_maps = []
    for d in range(N_CORES):
        t, h = divmod(d, 2)
        rows = indices[t, h * O_PER:(h + 1) * O_PER]
        c = codewords[rows]                                   # [512, 4096]
        ct = np.ascontiguousarray(c.T) * scales[t]            # [4096, 512]
        in_maps.append({
            "xt": xt,
            "rot": np.ascontiguousarray(rotations[t]),
            "ct": ct.astype(np.float32),
        })

    runner.put_inputs(in_maps)
    results = runner.run()

    full = np.concatenate([results[d]["out"] for d in range(N_CORES)], axis=1)
    full = full + bias[None, :]
    return full.reshape(4, 2048, D).astype(np.float32)


# revision 7
# speedup vs baseline: 11.1774x; 11.1774x over previous
"""DemopackDecoder Trainium2 kernel (8 NeuronCores, tensor-parallel).

Problem:
    weight = concat_t[ (codewords[indices[t]] @ rotations[t]) * scales[t] ]   # [4096, 4096]
    out    = x @ weight.T + bias                                              # [4, 2048, 4096]

Sharding: out_features (4096 = 4 tiles x 1024 rows) split across 8 cores,
512 rows each (core d -> tile t=d//2, half h=d%2). x is replicated; each core
computes its 512 output columns; host concatenates.

Per-core device program (all matmuls in float32r = full-speed fp32 PE mode):
  phase 1:  WT[e, r] = sum_d R[d, e] * CT[d, r]      (CT = scaled gathered
            codewords, transposed on host; R = rotation tile) -> WT resident
            in SBUF as [128, 32, 512]
  phase 2:  O[s, o] = sum_e XT[e, s] * WT[e, o]      (XT = x^T, streamed from
            HBM as stationary blocks; WT is the moving operand from SBUF)

Host does: transpose of x (layout prep), codeword gather + transpose + scale
fold (8.4 MB per core), bias add (bias is zeros in this problem, kept for
generality).
"""

import time

import numpy as np

import concourse.mybir as mybir
from concourse import bacc, tile

F32 = mybir.dt.float32
F32R = mybir.dt.float32r

D = 4096          # embed dim == in_features (contraction for both phases)
S = 8192          # B * S tokens
O_PER = 512       # out_features per core
N_CORES = 8

DO = D // 128     # 32 contraction chunks
P = 128

_CACHE = {}


def _build():
    nc = bacc.Bacc("TRN2", target_bir_lowering=False, debug=False,
                   num_devices=N_CORES)
    xt = nc.dram_tensor("xt", [D, S], F32R, kind="ExternalInput").ap()
    rot = nc.dram_tensor("rot", [D, D], F32R, kind="ExternalInput").ap()
    ct = nc.dram_tensor("ct", [D, O_PER], F32R, kind="ExternalInput").ap()
    out = nc.dram_tensor("out", [S, O_PER], F32, kind="ExternalOutput").ap()

    ct_r = ct.rearrange("(do p) r -> p do r", p=P)

    with tile.TileContext(nc) as tc:
        with (
            tc.tile_pool(name="resident", bufs=1) as resident,
            tc.tile_pool(name="rx", bufs=4) as rx,
            tc.tile_pool(name="outp", bufs=6) as outp,
            tc.tile_pool(name="ps", bufs=8, space="PSUM") as ps,
        ):
            ct_sb = resident.tile([P, DO, O_PER], F32R)
            wt_sb = resident.tile([P, DO, O_PER], F32R)

            for do in range(DO):
                nc.sync.dma_start(out=ct_sb[:, do, :], in_=ct_r[:, do, :])

            # ---- phase 1: WT = R^T-blocks x CT  (out e-partitions) ----
            for eg in range(8):          # groups of 4 e-tiles of 128
                psums = [
                    ps.tile([P, O_PER], F32, name=f"ps1_{eg}_{j}", tag="ps")
                    for j in range(4)
                ]
                for do in range(DO):
                    rt = rx.tile([P, 512], F32R, name="rt", tag="rt", bufs=4)
                    nc.sync.dma_start(
                        out=rt[:],
                        in_=rot[do * P:(do + 1) * P, eg * 512:(eg + 1) * 512],
                    )
                    for j in range(4):
                        nc.tensor.matmul(
                            psums[j][:],
                            lhsT=rt[:, j * P:(j + 1) * P],
                            rhs=ct_sb[:, do, :],
                            start=(do == 0),
                            stop=(do == DO - 1),
                        )
                for j in range(4):
                    nc.scalar.copy(wt_sb[:, eg * 4 + j, :], psums[j][:])

            # ---- phase 2: O = XT-blocks x WT  (out s-partitions) ----
            for sg in range(16):         # groups of 4 s-tiles of 128
                psums = [
                    ps.tile([P, O_PER], F32, name=f"ps2_{sg}_{j}", tag="ps")
                    for j in range(4)
                ]
                for eo in range(DO):
                    xtl = rx.tile([P, 512], F32R, name="xtl", tag="xtl", bufs=6)
                    nc.sync.dma_start(
                        out=xtl[:],
                        in_=xt[eo * P:(eo + 1) * P, sg * 512:(sg + 1) * 512],
                    )
                    for j in range(4):
                        nc.tensor.matmul(
                            psums[j][:],
                            lhsT=xtl[:, j * P:(j + 1) * P],
                            rhs=wt_sb[:, eo, :],
                            start=(eo == 0),
                            stop=(eo == DO - 1),
                        )
                for j in range(4):
                    st = sg * 4 + j
                    ot = outp.tile([P, O_PER], F32, name="ot", tag="ot")
                    nc.scalar.copy(ot[:], psums[j][:])
                    nc.sync.dma_start(
                        out=out[st * P:(st + 1) * P, :], in_=ot[:]
                    )

    nc.compile()
    return nc


class _Runner:
    """Compile once; execute the SPMD NEFF via PJRT shard_map repeatedly."""

    def __init__(self):
        import jax
        from jax.experimental.shard_map import shard_map
        from jax.sharding import Mesh, NamedSharding, PartitionSpec

        from concourse.bass2jax import (
            _bass_exec_p,
            install_neuronx_cc_hook,
            partition_id_tensor,
        )

        self.jax = jax
        install_neuronx_cc_hook()
        nc = _build()
        self.nc = nc

        in_names: list[str] = []
        out_names: list[str] = []
        out_avals: list = []
        zero_shapes: list = []
        partition_name = (
            nc.partition_id_tensor.name if nc.partition_id_tensor else None
        )
        for alloc in nc.m.functions[0].allocations:
            if not isinstance(alloc, mybir.MemoryLocationSet):
                continue
            name = alloc.memorylocations[0].name
            if alloc.kind == "ExternalInput":
                if name != partition_name:
                    in_names.append(name)
            elif alloc.kind == "ExternalOutput":
                np_dt = mybir.dt.np(alloc.dtype)
                out_names.append(name)
                out_avals.append(
                    jax.core.ShapedArray(tuple(alloc.tensor_shape), np_dt)
                )
                zero_shapes.append((tuple(alloc.tensor_shape), np_dt))

        self.n_params = len(in_names)
        self.in_names = list(in_names)
        self.out_names = list(out_names)
        self.out_avals = out_avals
        self.zero_shapes = zero_shapes

        all_in_names = in_names + out_names
        if partition_name is not None:
            all_in_names = all_in_names + [partition_name]

        def _body(*args):
            operands = list(args)
            if partition_name is not None:
                operands.append(partition_id_tensor())
            outs = _bass_exec_p.bind(
                *operands,
                out_avals=tuple(out_avals),
                in_names=tuple(all_in_names),
                out_names=tuple(out_names),
                lowering_input_output_aliases=(),
                sim_require_finite=True,
                sim_require_nnan=True,
                nc=nc,
            )
            return tuple(outs)

        devices = jax.devices()[:N_CORES]
        assert len(devices) == N_CORES
        self.mesh = Mesh(np.asarray(devices), ("core",))
        n_args = self.n_params + len(out_names)
        self.fn = jax.jit(
            shard_map(
                _body,
                mesh=self.mesh,
                in_specs=(PartitionSpec("core"),) * n_args,
                out_specs=(PartitionSpec("core"),) * len(out_names),
                check_rep=False,
            ),
            keep_unused=True,
        )
        self.sharding = NamedSharding(self.mesh, PartitionSpec("core"))
        self.dev_args = None

    def put_inputs(self, in_maps):
        jax = self.jax
        args = []
        for i, name in enumerate(self.in_names):
            cat = np.concatenate([np.asarray(m[name]) for m in in_maps], axis=0)
            args.append(jax.device_put(cat, self.sharding))
        for shape, np_dt in self.zero_shapes:
            z = np.zeros((N_CORES * shape[0], *shape[1:]), np_dt)
            args.append(jax.device_put(z, self.sharding))
        self.dev_args = args

    def run(self):
        jax = self.jax
        outs = self.fn(*self.dev_args)
        jax.block_until_ready(outs)
        res = []
        for c in range(N_CORES):
            res.append({
                name: np.asarray(outs[i]).reshape(
                    N_CORES, *self.out_avals[i].shape
                )[c]
                for i, name in enumerate(self.out_names)
            })
        return res

    def bench(self, iters=10):
        jax = self.jax
        outs = self.fn(*self.dev_args)
        jax.block_until_ready(outs)
        t0 = time.perf_counter()
        for _ in range(iters):
            outs = self.fn(*self.dev_args)
        jax.block_until_ready(outs)
        dt = (time.perf_counter() - t0) / iters
        return dt


def _get_runner():
    if "runner" not in _CACHE:
        _CACHE["runner"] = _Runner()
    return _CACHE["runner"]


def _run_resilient(in_maps):
    """Execute with retries: transient axon/NRT faults (device unrecoverable)
    have been observed; re-putting inputs and re-executing usually succeeds.
    As a last resort rebuild the runner (fresh executable)."""
    last_exc = None
    for attempt in range(4):
        try:
            runner = _get_runner()
            runner.put_inputs(in_maps)
            return runner.run()
        except Exception as e:  # noqa: BLE001 - retry any runtime fault
            last_exc = e
            _CACHE.pop("runner", None)
            time.sleep(2.0 * (attempt + 1))
    raise last_exc


def kernel(x, codewords, indices, rotations, scales, bias):
    x = np.asarray(x, dtype=np.float32)
    codewords = np.asarray(codewords, dtype=np.float32)
    indices = np.asarray(indices)
    rotations = np.asarray(rotations, dtype=np.float32)
    scales = np.asarray(scales, dtype=np.float32)
    bias = np.asarray(bias, dtype=np.float32)

    runner = _get_runner()

    xt = np.ascontiguousarray(x.reshape(S, D).T)  # [D, S]

    in_maps = []
    for d in range(N_CORES):
        t, h = divmod(d, 2)
        rows = indices[t, h * O_PER:(h + 1) * O_PER]
        c = codewords[rows]                                   # [512, 4096]
        ct = np.ascontiguousarray(c.T) * scales[t]            # [4096, 512]
        in_maps.append({
            "xt": xt,
            "rot": np.ascontiguousarray(rotations[t]),
            "ct": ct.astype(np.float32),
        })

    results = _run_resilient(in_maps)

    full = np.concatenate([results[d]["out"] for d in range(N_CORES)], axis=1)
    full = full + bias[None, :]
    return full.reshape(4, 2048, D).astype(np.float32)


# revision 16
# speedup vs baseline: 17.7203x; 1.5854x over previous
"""DemopackDecoder Trainium2 kernel (8 NeuronCores, tensor-parallel).

Problem:
    weight = concat_t[ (codewords[indices[t]] @ rotations[t]) * scales[t] ]   # [4096, 4096]
    out    = x @ weight.T + bias                                              # [4, 2048, 4096]

Sharding: out_features (4096 = 4 tiles x 1024 rows) split across 8 cores,
512 rows each (core d -> tile t=d//2, half h=d%2). x is replicated; each core
computes its 512 output columns; host concatenates.

Per-core device program (all matmuls in float32r = full-speed fp32 PE mode):
  phase 1:  WT[e, r] = sum_d R[d, e] * CT[d, r]      (CT = scaled gathered
            codewords, transposed on host; R = rotation tile) -> WT resident
            in SBUF as [128, 32, 512]
  phase 2:  O[s, o] = sum_e XT[e, s] * WT[e, o]      (XT = x^T, streamed from
            HBM as stationary blocks; WT is the moving operand from SBUF)

Host does: transpose of x (layout prep), codeword gather + transpose + scale
fold (8.4 MB per core), bias add (bias is zeros in this problem, kept for
generality).
"""

import time

import numpy as np

import concourse.mybir as mybir
from concourse import bacc, tile

F32 = mybir.dt.float32
F32R = mybir.dt.float32r

D = 4096          # embed dim == in_features (contraction for both phases)
S = 8192          # B * S tokens
O_PER = 512       # out_features per core
N_CORES = 8

DO = D // 128     # 32 contraction chunks
P = 128

_CACHE = {}


def _build():
    nc = bacc.Bacc("TRN2", target_bir_lowering=False, debug=False,
                   num_devices=N_CORES)
    xt = nc.dram_tensor("xt", [D, S], F32R, kind="ExternalInput").ap()
    rot = nc.dram_tensor("rot", [D, D], F32R, kind="ExternalInput").ap()
    ct = nc.dram_tensor("ct", [D, O_PER], F32R, kind="ExternalInput").ap()
    out = nc.dram_tensor("out", [S, O_PER], F32, kind="ExternalOutput").ap()

    ct_r = ct.rearrange("(do p) r -> p do r", p=P)

    with tile.TileContext(nc) as tc:
        with (
            tc.tile_pool(name="resident", bufs=1) as resident,
            tc.tile_pool(name="rx", bufs=4) as rx,
            tc.tile_pool(name="outp", bufs=8) as outp,
            tc.tile_pool(name="ps", bufs=8, space="PSUM") as ps,
        ):
            ct_sb = resident.tile([P, DO, O_PER], F32R)
            wt_sb = resident.tile([P, DO, O_PER], F32R)

            for do in range(DO):
                nc.sync.dma_start(out=ct_sb[:, do, :], in_=ct_r[:, do, :])

            # ---- phase 1: WT = R^T-blocks x CT  (out e-partitions) ----
            for eg in range(8):          # groups of 4 e-tiles of 128
                psums = [
                    ps.tile([P, O_PER], F32, name=f"ps1_{eg}_{j}", tag="ps")
                    for j in range(4)
                ]
                for do in range(DO):
                    rt = rx.tile([P, 512], F32R, name="rt", tag="rt", bufs=8)
                    nc.sync.dma_start(
                        out=rt[:],
                        in_=rot[do * P:(do + 1) * P, eg * 512:(eg + 1) * 512],
                    )
                    for j in range(4):
                        nc.tensor.matmul(
                            psums[j][:],
                            lhsT=rt[:, j * P:(j + 1) * P],
                            rhs=ct_sb[:, do, :],
                            start=(do == 0),
                            stop=(do == DO - 1),
                        )
                for j in range(4):
                    nc.scalar.copy(wt_sb[:, eg * 4 + j, :], psums[j][:])

            # ---- phase 2: O = XT-blocks x WT  (out s-partitions) ----
            for sg in range(16):         # groups of 4 s-tiles of 128
                psums = [
                    ps.tile([P, O_PER], F32, name=f"ps2_{sg}_{j}", tag="ps")
                    for j in range(4)
                ]
                for eo in range(DO):
                    xtl = rx.tile([P, 512], F32R, name="xtl", tag="xtl", bufs=12)
                    nc.sync.dma_start(
                        out=xtl[:],
                        in_=xt[eo * P:(eo + 1) * P, sg * 512:(sg + 1) * 512],
                    )
                    for j in range(4):
                        nc.tensor.matmul(
                            psums[j][:],
                            lhsT=xtl[:, j * P:(j + 1) * P],
                            rhs=wt_sb[:, eo, :],
                            start=(eo == 0),
                            stop=(eo == DO - 1),
                        )
                for j in range(4):
                    st = sg * 4 + j
                    ot = outp.tile([P, O_PER], F32, name="ot", tag="ot")
                    nc.scalar.copy(ot[:], psums[j][:])
                    nc.gpsimd.dma_start(
                        out=out[st * P:(st + 1) * P, :], in_=ot[:]
                    )

    nc.compile()
    return nc


class _Runner:
    """Compile once; execute the SPMD NEFF via PJRT shard_map repeatedly."""

    def __init__(self):
        import jax
        from jax.experimental.shard_map import shard_map
        from jax.sharding import Mesh, NamedSharding, PartitionSpec

        from concourse.bass2jax import (
            _bass_exec_p,
            install_neuronx_cc_hook,
            partition_id_tensor,
        )

        self.jax = jax
        install_neuronx_cc_hook()
        nc = _build()
        self.nc = nc

        in_names: list[str] = []
        out_names: list[str] = []
        out_avals: list = []
        zero_shapes: list = []
        partition_name = (
            nc.partition_id_tensor.name if nc.partition_id_tensor else None
        )
        for alloc in nc.m.functions[0].allocations:
            if not isinstance(alloc, mybir.MemoryLocationSet):
                continue
            name = alloc.memorylocations[0].name
            if alloc.kind == "ExternalInput":
                if name != partition_name:
                    in_names.append(name)
            elif alloc.kind == "ExternalOutput":
                np_dt = mybir.dt.np(alloc.dtype)
                out_names.append(name)
                out_avals.append(
                    jax.core.ShapedArray(tuple(alloc.tensor_shape), np_dt)
                )
                zero_shapes.append((tuple(alloc.tensor_shape), np_dt))

        self.n_params = len(in_names)
        self.in_names = list(in_names)
        self.out_names = list(out_names)
        self.out_avals = out_avals
        self.zero_shapes = zero_shapes

        all_in_names = in_names + out_names
        if partition_name is not None:
            all_in_names = all_in_names + [partition_name]

        def _body(*args):
            operands = list(args)
            if partition_name is not None:
                operands.append(partition_id_tensor())
            outs = _bass_exec_p.bind(
                *operands,
                out_avals=tuple(out_avals),
                in_names=tuple(all_in_names),
                out_names=tuple(out_names),
                lowering_input_output_aliases=(),
                sim_require_finite=True,
                sim_require_nnan=True,
                nc=nc,
            )
            return tuple(outs)

        devices = jax.devices()[:N_CORES]
        assert len(devices) == N_CORES
        self.mesh = Mesh(np.asarray(devices), ("core",))
        n_args = self.n_params + len(out_names)
        self.fn = jax.jit(
            shard_map(
                _body,
                mesh=self.mesh,
                in_specs=(PartitionSpec("core"),) * n_args,
                out_specs=(PartitionSpec("core"),) * len(out_names),
                check_rep=False,
            ),
            keep_unused=True,
        )
        self.sharding = NamedSharding(self.mesh, PartitionSpec("core"))
        self.dev_args = None

    def put_inputs(self, in_maps):
        jax = self.jax
        devices = list(self.mesh.devices.flat)
        args = []
        for name in self.in_names:
            per = [np.asarray(m[name]) for m in in_maps]
            gshape = (N_CORES * per[0].shape[0], *per[0].shape[1:])
            shards = [jax.device_put(per[c], devices[c]) for c in range(N_CORES)]
            args.append(jax.make_array_from_single_device_arrays(
                gshape, self.sharding, shards))
        for shape, np_dt in self.zero_shapes:
            z = np.zeros(shape, np_dt)
            shards = [jax.device_put(z, devices[c]) for c in range(N_CORES)]
            args.append(jax.make_array_from_single_device_arrays(
                (N_CORES * shape[0], *shape[1:]), self.sharding, shards))
        self.dev_args = args

    def run(self):
        jax = self.jax
        outs = self.fn(*self.dev_args)
        jax.block_until_ready(outs)
        res = []
        for c in range(N_CORES):
            res.append({
                name: np.asarray(outs[i]).reshape(
                    N_CORES, *self.out_avals[i].shape
                )[c]
                for i, name in enumerate(self.out_names)
            })
        return res

    def bench(self, iters=10):
        jax = self.jax
        outs = self.fn(*self.dev_args)
        jax.block_until_ready(outs)
        t0 = time.perf_counter()
        for _ in range(iters):
            outs = self.fn(*self.dev_args)
        jax.block_until_ready(outs)
        dt = (time.perf_counter() - t0) / iters
        return dt


def _get_runner():
    if "runner" not in _CACHE:
        _CACHE["runner"] = _Runner()
    return _CACHE["runner"]


def _run_resilient(in_maps):
    """Execute with retries: transient axon/NRT faults (device unrecoverable)
    have been observed; re-putting inputs and re-executing usually succeeds.
    As a last resort rebuild the runner (fresh executable)."""
    last_exc = None
    for attempt in range(4):
        try:
            runner = _get_runner()
            runner.put_inputs(in_maps)
            return runner.run()
        except Exception as e:  # noqa: BLE001 - retry any runtime fault
            last_exc = e
            _CACHE.pop("runner", None)
            time.sleep(2.0 * (attempt + 1))
    raise last_exc


def kernel(x, codewords, indices, rotations, scales, bias):
    x = np.asarray(x, dtype=np.float32)
    codewords = np.asarray(codewords, dtype=np.float32)
    indices = np.asarray(indices)
    rotations = np.asarray(rotations, dtype=np.float32)
    scales = np.asarray(scales, dtype=np.float32)
    bias = np.asarray(bias, dtype=np.float32)

    _get_runner()  # build + compile the executable up front (cached)

    xt = np.ascontiguousarray(x.reshape(S, D).T)  # [D, S]

    in_maps = []
    for d in range(N_CORES):
        t, h = divmod(d, 2)
        rows = indices[t, h * O_PER:(h + 1) * O_PER]
        c = codewords[rows]                                   # [512, 4096]
        ct = np.ascontiguousarray(c.T) * scales[t]            # [4096, 512]
        in_maps.append({
            "xt": xt,
            "rot": np.ascontiguousarray(rotations[t]),
            "ct": ct.astype(np.float32),
        })

    results = _run_resilient(in_maps)

    full = np.concatenate([results[d]["out"] for d in range(N_CORES)], axis=1)
    full = full + bias[None, :]
    return full.reshape(4, 2048, D).astype(np.float32)


# revision 21
# speedup vs baseline: 18.3856x; 1.0375x over previous
"""DemopackDecoder Trainium2 kernel (8 NeuronCores, tensor-parallel).

Problem:
    weight = concat_t[ (codewords[indices[t]] @ rotations[t]) * scales[t] ]   # [4096, 4096]
    out    = x @ weight.T + bias                                              # [4, 2048, 4096]

Sharding: out_features (4096 = 4 tiles x 1024 rows) split across 8 cores,
512 rows each (core d -> tile t=d//2, half h=d%2). x is replicated; each core
computes its 512 output columns; host concatenates.

Per-core device program (all matmuls in float32r = full-speed fp32 PE mode):
  phase 1:  WT[e, r] = sum_d R[d, e] * CT[d, r]      (CT = scaled gathered
            codewords, transposed on host; R = rotation tile) -> WT resident
            in SBUF as [128, 32, 512]
  phase 2:  O[s, o] = sum_e XT[e, s] * WT[e, o]      (XT = x^T, streamed from
            HBM as stationary blocks; WT is the moving operand from SBUF)

Host does: transpose of x (layout prep), codeword gather + transpose + scale
fold (8.4 MB per core), bias add (bias is zeros in this problem, kept for
generality).
"""

import time

import numpy as np

import concourse.mybir as mybir
from concourse import bacc, tile

F32 = mybir.dt.float32
F32R = mybir.dt.float32r

D = 4096          # embed dim == in_features (contraction for both phases)
S = 8192          # B * S tokens
O_PER = 512       # out_features per core
N_CORES = 8

DO = D // 128     # 32 contraction chunks
P = 128

_CACHE = {}


def _build():
    nc = bacc.Bacc("TRN2", target_bir_lowering=False, debug=False,
                   num_devices=N_CORES)
    xt = nc.dram_tensor("xt", [D, S], F32R, kind="ExternalInput").ap()
    rot = nc.dram_tensor("rot", [D, D], F32R, kind="ExternalInput").ap()
    ct = nc.dram_tensor("ct", [D, O_PER], F32R, kind="ExternalInput").ap()
    out = nc.dram_tensor("out", [S, O_PER], F32, kind="ExternalOutput").ap()

    ct_r = ct.rearrange("(do p) r -> p do r", p=P)

    with tile.TileContext(nc) as tc:
        with (
            tc.tile_pool(name="resident", bufs=1) as resident,
            tc.tile_pool(name="rx", bufs=4) as rx,
            tc.tile_pool(name="outp", bufs=8) as outp,
            tc.tile_pool(name="ps", bufs=8, space="PSUM") as ps,
        ):
            ct_sb = resident.tile([P, DO, O_PER], F32R)
            wt_sb = resident.tile([P, DO, O_PER], F32R)

            for do in range(DO):
                nc.sync.dma_start(out=ct_sb[:, do, :], in_=ct_r[:, do, :])

            # ---- phase 1: WT = R^T-blocks x CT  (out e-partitions) ----
            for eg in range(8):          # groups of 4 e-tiles of 128
                psums = [
                    ps.tile([P, O_PER], F32, name=f"ps1_{eg}_{j}", tag="ps")
                    for j in range(4)
                ]
                for do in range(DO):
                    rt = rx.tile([P, 512], F32R, name="rt", tag="rt", bufs=8)
                    nc.sync.dma_start(
                        out=rt[:],
                        in_=rot[do * P:(do + 1) * P, eg * 512:(eg + 1) * 512],
                    )
                    for j in range(4):
                        nc.tensor.matmul(
                            psums[j][:],
                            lhsT=rt[:, j * P:(j + 1) * P],
                            rhs=ct_sb[:, do, :],
                            start=(do == 0),
                            stop=(do == DO - 1),
                        )
                for j in range(4):
                    nc.scalar.copy(wt_sb[:, eg * 4 + j, :], psums[j][:])

            # ---- phase 2: O = XT-blocks x WT  (out s-partitions) ----
            for sg in range(16):         # groups of 4 s-tiles of 128
                psums = [
                    ps.tile([P, O_PER], F32, name=f"ps2_{sg}_{j}", tag="ps")
                    for j in range(4)
                ]
                for eo in range(DO):
                    xtl = rx.tile([P, 512], F32R, name="xtl", tag="xtl", bufs=12)
                    nc.sync.dma_start(
                        out=xtl[:],
                        in_=xt[eo * P:(eo + 1) * P, sg * 512:(sg + 1) * 512],
                    )
                    for j in range(4):
                        nc.tensor.matmul(
                            psums[j][:],
                            lhsT=xtl[:, j * P:(j + 1) * P],
                            rhs=wt_sb[:, eo, :],
                            start=(eo == 0),
                            stop=(eo == DO - 1),
                        )
                for j in range(4):
                    st = sg * 4 + j
                    ot = outp.tile([P, O_PER], F32, name="ot", tag="ot")
                    nc.scalar.copy(ot[:], psums[j][:])
                    nc.gpsimd.dma_start(
                        out=out[st * P:(st + 1) * P, :], in_=ot[:]
                    )

    nc.compile()
    return nc


class _Runner:
    """Compile once; execute the SPMD NEFF via PJRT shard_map repeatedly."""

    def __init__(self):
        import jax
        from jax.experimental.shard_map import shard_map
        from jax.sharding import Mesh, NamedSharding, PartitionSpec

        from concourse.bass2jax import (
            _bass_exec_p,
            install_neuronx_cc_hook,
            partition_id_tensor,
        )

        self.jax = jax
        install_neuronx_cc_hook()
        nc = _build()
        self.nc = nc

        in_names: list[str] = []
        out_names: list[str] = []
        out_avals: list = []
        zero_shapes: list = []
        partition_name = (
            nc.partition_id_tensor.name if nc.partition_id_tensor else None
        )
        for alloc in nc.m.functions[0].allocations:
            if not isinstance(alloc, mybir.MemoryLocationSet):
                continue
            name = alloc.memorylocations[0].name
            if alloc.kind == "ExternalInput":
                if name != partition_name:
                    in_names.append(name)
            elif alloc.kind == "ExternalOutput":
                np_dt = mybir.dt.np(alloc.dtype)
                out_names.append(name)
                out_avals.append(
                    jax.core.ShapedArray(tuple(alloc.tensor_shape), np_dt)
                )
                zero_shapes.append((tuple(alloc.tensor_shape), np_dt))

        self.n_params = len(in_names)
        self.in_names = list(in_names)
        self.out_names = list(out_names)
        self.out_avals = out_avals
        self.zero_shapes = zero_shapes

        all_in_names = in_names + out_names
        if partition_name is not None:
            all_in_names = all_in_names + [partition_name]

        def _body(*args):
            operands = list(args)
            if partition_name is not None:
                operands.append(partition_id_tensor())
            outs = _bass_exec_p.bind(
                *operands,
                out_avals=tuple(out_avals),
                in_names=tuple(all_in_names),
                out_names=tuple(out_names),
                lowering_input_output_aliases=(),
                sim_require_finite=True,
                sim_require_nnan=True,
                nc=nc,
            )
            return tuple(outs)

        devices = jax.devices()[:N_CORES]
        assert len(devices) == N_CORES
        self.mesh = Mesh(np.asarray(devices), ("core",))
        n_args = self.n_params + len(out_names)
        self.fn = jax.jit(
            shard_map(
                _body,
                mesh=self.mesh,
                in_specs=(PartitionSpec("core"),) * n_args,
                out_specs=(PartitionSpec("core"),) * len(out_names),
                check_rep=False,
            ),
            keep_unused=True,
        )
        self.sharding = NamedSharding(self.mesh, PartitionSpec("core"))
        self.dev_args = None

    def put_inputs(self, in_maps):
        jax = self.jax
        devices = list(self.mesh.devices.flat)
        args = []
        for name in self.in_names:
            per = [np.asarray(m[name]) for m in in_maps]
            gshape = (N_CORES * per[0].shape[0], *per[0].shape[1:])
            shards = [jax.device_put(per[c], devices[c]) for c in range(N_CORES)]
            args.append(jax.make_array_from_single_device_arrays(
                gshape, self.sharding, shards))
        for shape, np_dt in self.zero_shapes:
            z = np.zeros(shape, np_dt)
            shards = [jax.device_put(z, devices[c]) for c in range(N_CORES)]
            args.append(jax.make_array_from_single_device_arrays(
                (N_CORES * shape[0], *shape[1:]), self.sharding, shards))
        self.dev_args = args

    def run(self):
        jax = self.jax
        outs = self.fn(*self.dev_args)
        jax.block_until_ready(outs)
        res = []
        for c in range(N_CORES):
            res.append({
                name: np.asarray(outs[i]).reshape(
                    N_CORES, *self.out_avals[i].shape
                )[c]
                for i, name in enumerate(self.out_names)
            })
        return res

    def bench(self, iters=10):
        jax = self.jax
        outs = self.fn(*self.dev_args)
        jax.block_until_ready(outs)
        t0 = time.perf_counter()
        for _ in range(iters):
            outs = self.fn(*self.dev_args)
        jax.block_until_ready(outs)
        dt = (time.perf_counter() - t0) / iters
        return dt


def _get_runner():
    if "runner" not in _CACHE:
        _CACHE["runner"] = _Runner()
    return _CACHE["runner"]


def _run_resilient(in_maps):
    """Execute with retries: transient axon/NRT faults (device unrecoverable)
    have been observed; re-putting inputs and re-executing usually succeeds.
    As a last resort rebuild the runner (fresh executable)."""
    last_exc = None
    for attempt in range(4):
        try:
            runner = _get_runner()
            runner.put_inputs(in_maps)
            return runner.run()
        except Exception as e:  # noqa: BLE001 - retry any runtime fault
            last_exc = e
            _CACHE.pop("runner", None)
            time.sleep(2.0 * (attempt + 1))
    raise last_exc


def kernel(x, codewords, indices, rotations, scales, bias):
    x = np.asarray(x, dtype=np.float32)
    codewords = np.asarray(codewords, dtype=np.float32)
    indices = np.asarray(indices)
    rotations = np.asarray(rotations, dtype=np.float32)
    scales = np.asarray(scales, dtype=np.float32)
    bias = np.asarray(bias, dtype=np.float32)

    _get_runner()  # build + compile the executable up front (cached)

    xt = np.ascontiguousarray(x.reshape(S, D).T)  # [D, S]

    in_maps = []
    for d in range(N_CORES):
        t, h = divmod(d, 2)
        rows = indices[t, h * O_PER:(h + 1) * O_PER]
        c = codewords[rows]                                   # [512, 4096]
        ct = np.ascontiguousarray(c.T) * scales[t]            # [4096, 512]
        in_maps.append({
            "xt": xt,
            "rot": np.ascontiguousarray(rotations[t]),
            "ct": ct.astype(np.float32),
        })

    results = _run_resilient(in_maps)

    full = np.concatenate([results[d]["out"] for d in range(N_CORES)], axis=1)
    full = full + bias[None, :]
    return full.reshape(4, 2048, D).astype(np.float32)
